# revision 1
# baseline (speedup 1.0000x reference)
"""Trainium2 Bass kernel for the CoordPooling+SFP gate module.

Computation (per batch b):
  y_pre = [sum_w x | sum_h x] / 64            [C, H+W]   (C=384, H=W=64)
  y  = relu((Wy/64 @ y_pre_sums) * sy + by)   [C, 128]
  xh = relu((Wh @ y[:, :64]) * sh + bh)       [C, 64]
  xw = relu((Ww @ y[:, 64:]) * sw + bw)       [C, 64]
  z_raw[c] = sum_L y[c, :]   (haar wavelet level-i approx mean == scaled row sum)
  z  = fc1(relu(bn1(fc0(z_raw * wscale))))    [C]
  out = x * sigmoid(xh outer xw) + x * z

Sharding: data-parallel over batch, 4 batches per core on 8 cores.

Engine split (per core): DVE does sum_w reduce + most of the outer
product + the fused (s+z)*x; GpSimd does sum_h (in-place add ladder) +
a slice of the outer product; ScalarE does sigmoid + all CBR epilogues
(with accumulate for the wavelet row-sums); TensorE does the matmuls in
float32r, batch-paired so the moving free dim reaches 256.
"""

import sys
import numpy as np

for _p in ("/opt/trn_rl_repo", "/root/.axon_site/_ro/trn_rl_repo"):
    if _p not in sys.path:
        sys.path.append(_p)

import concourse.bass as bass
import concourse.tile as tile
from concourse import bacc, mybir
from concourse.bass_utils import run_bass_kernel_spmd
from concourse.bass import _add_dep_helper

F32 = mybir.dt.float32
F32R = mybir.dt.float32r
AF = mybir.ActivationFunctionType
OP = mybir.AluOpType

N_CORES = 8
B, C, H, W = 32, 384, 64, 64
BS = B // N_CORES          # batches per core
P = 128                    # partitions
KC = C // P                # channel chunks (3)
R = 24                     # gate bottleneck
EPS = 1e-5

GPS_H = 8                  # rows of each 32-row half handled by GpSimd
X_BUFS = 3

# const blob layout (free-dim offsets within [128, CONST_F])
_OFF_WY = 0
_OFF_WH = _OFF_WY + KC * C      # 1152
_OFF_WW = _OFF_WH + KC * C
WBLOB_F = _OFF_WW + KC * C      # 3456 (separate fp32r blob)
_OFF_FC0 = 0
_OFF_SY = _OFF_FC0 + KC * R     # 72
_OFF_BY = _OFF_SY + KC
_OFF_SH = _OFF_BY + KC
_OFF_BH = _OFF_SH + KC
_OFF_SW = _OFF_BH + KC
_OFF_BW = _OFF_SW + KC
_OFF_FC1B = _OFF_BW + KC
CONST_F = _OFF_FC1B + KC        # 93

_ZOFF_FC1 = 0
_ZOFF_S = KC * P                # 384
_ZOFF_B = _ZOFF_S + 1
ZCONST_F = _ZOFF_B + 1          # 386

_compiled = None


def _build():
    nc = bacc.Bacc("TRN2", target_bir_lowering=False, debug=False,
                   num_devices=N_CORES)
    x_d = nc.dram_tensor("x", [BS, C, H, W], F32, kind="ExternalInput")
    wbl_d = nc.dram_tensor("wbl", [P, WBLOB_F], F32R, kind="ExternalInput")
    cst_d = nc.dram_tensor("cst", [P, CONST_F], F32, kind="ExternalInput")
    zcst_d = nc.dram_tensor("zcst", [R, ZCONST_F], F32, kind="ExternalInput")
    out_d = nc.dram_tensor("out", [BS, C, H, W], F32, kind="ExternalOutput")

    with tile.TileContext(nc) as tc:
        with (
            tc.tile_pool(name="consts", bufs=1) as consts,
            tc.tile_pool(name="xp", bufs=3) as xpool,
            tc.tile_pool(name="ypre", bufs=1) as ypre_pool,
            tc.tile_pool(name="ysb", bufs=1) as ysb_pool,
            tc.tile_pool(name="hwp", bufs=2) as hw_pool,
            tc.tile_pool(name="zp", bufs=2) as zpool,
            tc.tile_pool(name="tp", bufs=2) as tpool,
            tc.tile_pool(name="ladp", bufs=1) as ladder_pool,
            tc.tile_pool(name="psy", bufs=2, space=bass.MemorySpace.PSUM) as psy,
            tc.tile_pool(name="pshw", bufs=2, space=bass.MemorySpace.PSUM) as pshw,
            tc.tile_pool(name="psz", bufs=2, space=bass.MemorySpace.PSUM) as psz,
        ):
            wbl = consts.tile([P, WBLOB_F], F32R)
            nc.scalar.dma_start(wbl[:], wbl_d.ap())
            cst = consts.tile([P, CONST_F], F32)
            nc.scalar.dma_start(cst[:], cst_d.ap())
            zcst = consts.tile([R, ZCONST_F], F32)
            nc.scalar.dma_start(zcst[:], zcst_d.ap())

            wyT = wbl[:, _OFF_WY:_OFF_WH].rearrange("p (k o) -> p k o", k=KC)
            whT = wbl[:, _OFF_WH:_OFF_WW].rearrange("p (k o) -> p k o", k=KC)
            wwT = wbl[:, _OFF_WW:WBLOB_F].rearrange("p (k o) -> p k o", k=KC)
            fc0T = cst[:, _OFF_FC0:_OFF_SY].rearrange("p (k r) -> p k r", k=KC)
            sy_t = cst[:, _OFF_SY:_OFF_BY]
            by_t = cst[:, _OFF_BY:_OFF_SH]
            sh_t = cst[:, _OFF_SH:_OFF_BH]
            bh_t = cst[:, _OFF_BH:_OFF_SW]
            sw_t = cst[:, _OFF_SW:_OFF_BW]
            bw_t = cst[:, _OFF_BW:_OFF_FC1B]
            fc1b_t = cst[:, _OFF_FC1B:CONST_F]
            fc1T = zcst[:, _ZOFF_FC1:_ZOFF_S].rearrange("p (k o) -> p k o", k=KC)
            z2s_t = zcst[:, _ZOFF_S:_ZOFF_S + 1]
            z2b_t = zcst[:, _ZOFF_B:_ZOFF_B + 1]

            # pre-warm the sigmoid table set while the first loads are
            # in flight (relu/identity live in the same set, so this is
            # the only ACT_TABLE_LOAD and it is off the critical path)
            warm = consts.tile([P, 1], F32)
            nc.scalar.activation(warm[:], cst[:, 0:1], AF.Sigmoid)

            NH = 2                 # phase-2 h-halves per chunk
            HH = H // NH           # 16

            def phase2(st):
                # out = (sigmoid(xh outer xw) + z) * x, in place over x
                x_sb, xh, xw, z3, b = st
                for oc in range(KC):
                    for hh in range(NH):
                        h0 = hh * HH
                        t_t = tpool.tile([P, HH, W], F32, tag="t", name="t_t")
                        nc.vector.tensor_mul(
                            t_t[:],
                            xh[:, oc, h0:h0 + HH].unsqueeze(2)
                              .broadcast_to([P, HH, W]),
                            xw[:, oc, :].unsqueeze(1)
                              .broadcast_to([P, HH, W]))
                        nc.scalar.activation(t_t[:], t_t[:], AF.Sigmoid)
                        last = nc.vector.scalar_tensor_tensor(
                            x_sb[:, oc, h0:h0 + HH, :],
                            t_t[:], z3[:, oc, :],
                            x_sb[:, oc, h0:h0 + HH, :],
                            op0=OP.add, op1=OP.mult)
                    nc.sync.dma_start(
                        out_d.ap()[b, oc * P:(oc + 1) * P],
                        x_sb[:, oc, :, :])
                return last

            prev = None
            for b in range(BS):
                x_sb = xpool.tile([P, KC, H, W], F32, tag="x", name="xsb")
                xs = x_d.ap()[b].rearrange("(k p) h w -> p k h w", p=P)
                lad = ladder_pool.tile([P, KC, 32, W], F32, tag="lad")
                y_pre = ypre_pool.tile([P, KC, H + W], F32R, tag="ypre")
                for kc in range(KC):
                    nc.sync.dma_start(x_sb[:, kc, :, :], xs[:, kc, :, :])

                for kc in range(KC):
                    # sum over w: contiguous 1-input DVE reduce — 1-input
                    # DVE ops share no SBUF port with GpSimd, so they
                    # overlap the GpSimd folds cleanly
                    with nc.allow_low_precision("fp32r matmul input rounding"):
                        nc.vector.tensor_reduce(
                            y_pre[:, kc, 0:H], x_sb[:, kc, :, :],
                            axis=mybir.AxisListType.X, op=OP.add)
                    # sum over h, step 1: GpSimd folds 64 rows -> 32.
                    # Gated behind the previous phase 2 so the fold lands
                    # in the reduce window, not on top of DVE 2-input ops.
                    nc.gpsimd.tensor_add(
                        lad[:, kc, :, :], x_sb[:, kc, 0:32, :],
                        x_sb[:, kc, 32:64, :])

                if prev is not None and b < BS - 1:
                    phase2(prev)
                    prev = None

                # sum over h, step 2: DVE folds 32 -> 1 with contiguous
                # adds (GpSimd is idle again by now, so no port fights)
                nc.vector.tensor_add(
                    lad[:, :, 0:16, :], lad[:, :, 0:16, :],
                    lad[:, :, 16:32, :])
                nc.vector.tensor_add(
                    lad[:, :, 0:8, :], lad[:, :, 0:8, :], lad[:, :, 8:16, :])
                nc.vector.tensor_add(
                    lad[:, :, 0:4, :], lad[:, :, 0:4, :], lad[:, :, 4:8, :])
                nc.vector.tensor_add(
                    lad[:, :, 0:2, :], lad[:, :, 0:2, :], lad[:, :, 2:4, :])
                with nc.allow_low_precision("fp32r matmul input rounding"):
                    nc.vector.tensor_add(
                        y_pre[:, :, H:H + W], lad[:, :, 0, :], lad[:, :, 1, :])

                if prev is not None:
                    # last batch: run the previous phase 2 after this
                    # batch's ladder so its small compute overlaps it
                    phase2(prev)

                # small compute, z-chain first so z3/xh/xw arrive before
                # the next iteration's phase 2 needs them
                psum_y = psy.tile([P, KC, H + W], F32, tag="py")
                for oc in range(KC):
                    for kc in range(KC):
                        nc.tensor.matmul(
                            psum_y[:, oc, :],
                            wyT[:, kc, oc * P:(oc + 1) * P],
                            y_pre[:, kc, :],
                            start=(kc == 0), stop=(kc == KC - 1))
                y_sb = ysb_pool.tile([P, KC, H + W], F32R, tag="y")
                zraw = zpool.tile([P, KC, 1], F32, tag="zraw")
                for oc in range(KC):
                    nc.scalar.activation(
                        y_sb[:, oc, :], psum_y[:, oc, :], AF.Relu,
                        bias=by_t[:, oc:oc + 1], scale=sy_t[:, oc:oc + 1],
                        accum_out=zraw[:, oc, :])

                psum_z = psz.tile([R, 1], F32, tag="pz")
                for kc in range(KC):
                    nc.tensor.matmul(
                        psum_z[:], fc0T[:, kc, :], zraw[:, kc, :],
                        start=(kc == 0), stop=(kc == KC - 1))
                z2 = zpool.tile([R, 1], F32, tag="z2")
                nc.scalar.activation(z2[:], psum_z[:], AF.Relu,
                                     bias=z2b_t[:], scale=z2s_t[:])
                psum_z3 = psz.tile([P, KC], F32, tag="pz3")
                for oc in range(KC):
                    nc.tensor.matmul(
                        psum_z3[:, oc:oc + 1], fc1T[:, oc, :], z2[:],
                        start=True, stop=True)
                z3 = zpool.tile([P, KC, 1], F32, tag="z3")
                for oc in range(KC):
                    nc.scalar.activation(
                        z3[:, oc, :], psum_z3[:, oc:oc + 1], AF.Identity,
                        bias=fc1b_t[:, oc:oc + 1])

                psum_hw = pshw.tile([P, KC, H + W], F32, tag="phw")
                for oc in range(KC):
                    for kc in range(KC):
                        nc.tensor.matmul(
                            psum_hw[:, oc, 0:H],
                            whT[:, kc, oc * P:(oc + 1) * P],
                            y_sb[:, kc, 0:H],
                            start=(kc == 0), stop=(kc == KC - 1))
                    for kc in range(KC):
                        nc.tensor.matmul(
                            psum_hw[:, oc, H:H + W],
                            wwT[:, kc, oc * P:(oc + 1) * P],
                            y_sb[:, kc, H:H + W],
                            start=(kc == 0), stop=(kc == KC - 1))
                xh = hw_pool.tile([P, KC, H], F32, tag="xh")
                xw = hw_pool.tile([P, KC, W], F32, tag="xw")
                for oc in range(KC):
                    nc.scalar.activation(
                        xh[:, oc, :], psum_hw[:, oc, 0:H], AF.Relu,
                        bias=bh_t[:, oc:oc + 1], scale=sh_t[:, oc:oc + 1])
                    nc.scalar.activation(
                        xw[:, oc, :], psum_hw[:, oc, H:H + W], AF.Relu,
                        bias=bw_t[:, oc:oc + 1], scale=sw_t[:, oc:oc + 1])

                prev = (x_sb, xh, xw, z3, b)
            phase2(prev)

    nc.compile()
    return nc


def _pack_consts(Wy, gy, by, Wh, gh, bh, Ww, gw, bw,
                 fc0_w, fc0_b, bn1_g, bn1_b, fc1_w, fc1_b):
    inv = 1.0 / np.sqrt(1.0 + EPS)

    def chunked_T(w):
        # [out, in] -> lhsT tile [p, kc, out]
        return np.ascontiguousarray(
            w.T.reshape(KC, P, C).transpose(1, 0, 2))

    def lanes(v):
        # [C] -> [p, kc]
        return np.ascontiguousarray(v.reshape(KC, P).T)

    wbl = np.empty((P, WBLOB_F), np.float32)
    wbl[:, _OFF_WY:_OFF_WH] = chunked_T(Wy / 64.0).reshape(P, KC * C)
    wbl[:, _OFF_WH:_OFF_WW] = chunked_T(Wh).reshape(P, KC * C)
    wbl[:, _OFF_WW:WBLOB_F] = chunked_T(Ww).reshape(P, KC * C)
    cst = np.empty((P, CONST_F), np.float32)
    # wavelet level-i scale per channel chunk, folded into fc0
    wscale = np.repeat(2.0 ** (np.arange(1, KC + 1) / 2.0) / (H + W), P)
    fc0T_s = (fc0_w * wscale[None, :]).T        # [C, R]
    cst[:, _OFF_FC0:_OFF_SY] = fc0T_s.reshape(KC, P, R).transpose(1, 0, 2) \
                                     .reshape(P, KC * R)
    cst[:, _OFF_SY:_OFF_BY] = lanes(gy * inv)
    cst[:, _OFF_BY:_OFF_SH] = lanes(by)
    cst[:, _OFF_SH:_OFF_BH] = lanes(gh * inv)
    cst[:, _OFF_BH:_OFF_SW] = lanes(bh)
    cst[:, _OFF_SW:_OFF_BW] = lanes(gw * inv)
    cst[:, _OFF_BW:_OFF_FC1B] = lanes(bw)
    cst[:, _OFF_FC1B:CONST_F] = lanes(fc1_b)

    zcst = np.empty((R, ZCONST_F), np.float32)
    zcst[:, _ZOFF_FC1:_ZOFF_S] = fc1_w.T.reshape(R, KC * P)
    z2s = bn1_g * inv
    zcst[:, _ZOFF_S] = z2s
    zcst[:, _ZOFF_B] = fc0_b * z2s + bn1_b
    return wbl, cst, zcst


def _get_compiled():
    global _compiled
    if _compiled is None:
        _compiled = _build()
    return _compiled


def kernel(x, Wy, gy, by, Wh, gh, bh, Ww, gw, bw,
           fc0_w, fc0_b, bn1_g, bn1_b, fc1_w, fc1_b,
           _trace=False, _trace_kwargs=None):
    nc = _get_compiled()
    wbl, cst, zcst = _pack_consts(
        np.asarray(Wy, np.float32), np.asarray(gy, np.float32),
        np.asarray(by, np.float32), np.asarray(Wh, np.float32),
        np.asarray(gh, np.float32), np.asarray(bh, np.float32),
        np.asarray(Ww, np.float32), np.asarray(gw, np.float32),
        np.asarray(bw, np.float32), np.asarray(fc0_w, np.float32),
        np.asarray(fc0_b, np.float32), np.asarray(bn1_g, np.float32),
        np.asarray(bn1_b, np.float32), np.asarray(fc1_w, np.float32),
        np.asarray(fc1_b, np.float32))
    x = np.ascontiguousarray(np.asarray(x, np.float32))
    in_maps = [
        {"x": x[i * BS:(i + 1) * BS], "wbl": wbl, "cst": cst, "zcst": zcst}
        for i in range(N_CORES)
    ]
    res = run_bass_kernel_spmd(
        nc, in_maps, list(range(N_CORES)),
        trace=_trace, **(_trace_kwargs or {}))
    out = np.concatenate([res.results[i]["out"] for i in range(N_CORES)],
                         axis=0)
    if _trace:
        return out, res
    return out

